# revision 4
# baseline (speedup 1.0000x reference)
"""Trainium2 Bass kernel for nn_EntityAttentionLayer (sparse attention).

Strategy (8 cores, data-parallel over bs):
  - Host side: shard bs across 8 cores (64 items each), pre-transpose
    entities to E^T[in_dim, ne] per batch, convert masks to
    multiplicative keep-masks.
  - K/V/Q projections run on the PE in fp8-e4m3 DoubleRow mode
    (2 contraction planes per instruction, 0.5 cycles/row). Accuracy is
    recovered with an asymmetric 3-term hi/lo split, all partial
    products at scale 1 so they share one PSUM accumulation group:
        E @ W ~= E_hi@W_hi + E_lo@W_hi + (E_hi/32)@(32*W_lo)
    (E_lo is representable in e4m3 directly; W_lo underflows, so it is
    carried at x32 against an exactly exponent-shifted E_hi/32.)
  - On chip, per batch b (processed in pairs, Q in octets of 8):
      K^T[ed, ne], V[ne, ed], Q^T[ed, q]   fp8 DR 3-term matmuls
      logits^T[ne, q] per head             bf16, lhsT=K^T_h, rhs=Q^T_h
      wm = exp(logits * 1/sqrt(hd))        on ACT (scale folded in)
      wm *= keep^T                         on GPSIMD (h-broadcast)
      wmsum = wm(n2=0)+wm(n2=1)            on GPSIMD
      sums broadcast [128, h*q]            single PE matmul w/ ones lhsT
      attn^T unnorm [2-heads, hp*b*q]      bf16 lhsT=V slices, rhs=wm
      attn = attn_unnorm * 1/sums          DVE (approx recip + muls)
      out[b*q, out] = attn^T.T @ W_out (+bias & post-mask fused on DVE)
      DMA out.
  Attention + out-proj matmuls bf16, fp32 PSUM accumulation.
"""

import numpy as np
import ml_dtypes

BS, NE, NQ, IN_DIM, ED, OUT_DIM, H, HD = 512, 256, 64, 512, 512, 512, 8, 64
NCORES = 8
BPC = BS // NCORES          # 64 batches per core
OCT = 8                     # batches per super-batch (Q^T amortization)
NOCT = BPC // OCT           # 8
PAIRS_PER_OCT = OCT // 2    # 4
NPAIRS = BPC // 2           # 32
SCALE = 1.0 / float(np.sqrt(HD))
LO_S = 32.0                 # residual upscale for the W_lo fp8 term

BF16 = ml_dtypes.bfloat16
F8 = ml_dtypes.float8_e4m3fn

_BUILT = {}
LAST_RESULT = None


def _build_nc():
    import concourse.tile as tile
    from concourse import bacc, mybir
    from contextlib import ExitStack

    f32 = mybir.dt.float32
    bf16 = mybir.dt.bfloat16
    f8 = mybir.dt.float8e4
    DR = mybir.MatmulPerfMode.DoubleRow

    nc = bacc.Bacc("TRN2", target_bir_lowering=False)

    e8h_d = nc.dram_tensor("e8h", [NOCT, 128, 2, 2, OCT, NE], f8, kind="ExternalInput")
    e8l_d = nc.dram_tensor("e8l", [NOCT, 128, 2, 2, OCT, NE], f8, kind="ExternalInput")
    e8s_d = nc.dram_tensor("e8s", [NOCT, 128, 2, 2, OCT, NE], f8, kind="ExternalInput")
    q8h_d = nc.dram_tensor("q8h", [NOCT, 128, 2, 2, OCT, NQ], f8, kind="ExternalInput")
    q8l_d = nc.dram_tensor("q8l", [NOCT, 128, 2, 2, OCT, NQ], f8, kind="ExternalInput")
    q8s_d = nc.dram_tensor("q8s", [NOCT, 128, 2, 2, OCT, NQ], f8, kind="ExternalInput")
    keep_d = nc.dram_tensor("keep", [NOCT, 128, OCT, 2, NQ], bf16, kind="ExternalInput")
    postm_d = nc.dram_tensor("postm", [128, NPAIRS], f32, kind="ExternalInput")
    w8h_d = nc.dram_tensor("w8h", [128, 2, 2, 3 * ED], f8, kind="ExternalInput")
    w8ls_d = nc.dram_tensor("w8ls", [128, 2, 2, 3 * ED], f8, kind="ExternalInput")
    w_out_d = nc.dram_tensor("w_out", [4, 128, OUT_DIM], bf16, kind="ExternalInput")
    b_out_d = nc.dram_tensor("b_out", [1, OUT_DIM], f32, kind="ExternalInput")
    out_d = nc.dram_tensor("out", [BPC, NQ, OUT_DIM], f32, kind="ExternalOutput")

    with ExitStack() as ctx:
        tc = ctx.enter_context(tile.TileContext(nc))
        consts = ctx.enter_context(tc.tile_pool(name="consts", bufs=1))
        p_e8 = ctx.enter_context(tc.tile_pool(name="p_e8", bufs=3))
        p_keep = ctx.enter_context(tc.tile_pool(name="p_keep", bufs=3))
        p_kT = ctx.enter_context(tc.tile_pool(name="p_kT", bufs=3))
        p_v = ctx.enter_context(tc.tile_pool(name="p_v", bufs=3))
        p_wm = ctx.enter_context(tc.tile_pool(name="p_wm", bufs=6))
        p_wmsum = ctx.enter_context(tc.tile_pool(name="p_wmsum", bufs=3))
        p_recip = ctx.enter_context(tc.tile_pool(name="p_recip", bufs=2))
        p_attn = ctx.enter_context(tc.tile_pool(name="p_attn", bufs=2))
        p_out = ctx.enter_context(tc.tile_pool(name="p_out", bufs=3))
        pp = ctx.enter_context(tc.tile_pool(name="pp", bufs=2, space="PSUM"))

        # Constants
        w8h_sb = consts.tile([128, 2, 2, 3 * ED], f8)
        nc.scalar.dma_start(out=w8h_sb, in_=w8h_d[:, :, :, :])
        w8ls_sb = consts.tile([128, 2, 2, 3 * ED], f8)
        nc.scalar.dma_start(out=w8ls_sb, in_=w8ls_d[:, :, :, :])
        wo_sb = consts.tile([128, 4, OUT_DIM], bf16)
        nc.gpsimd.dma_start(out=wo_sb, in_=w_out_d[:, :, :].rearrange("k p n -> p k n"))
        bias_bc = consts.tile([128, OUT_DIM], f32)
        nc.gpsimd.dma_start(out=bias_bc, in_=b_out_d[:, :].to_broadcast([128, OUT_DIM]))
        postm_sb = consts.tile([128, NPAIRS], f32)
        nc.gpsimd.dma_start(out=postm_sb, in_=postm_d[:, :])
        ones_sb = consts.tile([128, 128], bf16)
        nc.vector.memset(ones_sb, 1.0)
        # Persistent zero-padded Q^T tiles (manual double buffer by octet
        # parity). Layout [128, m, h2, b, q]: head parity h2 selects which
        # 64-row half holds data; the other half stays zero so logits
        # matmuls can use full K=128 operands at base partition 0
        # (operands at base partition 64 fault on HW).
        qz0 = consts.tile([128, 4, 2, OCT, HD], bf16)
        nc.vector.memset(qz0, 0.0)
        qz1 = consts.tile([128, 4, 2, OCT, HD], bf16)
        nc.vector.memset(qz1, 0.0)
        qz_bufs = [qz0, qz1]

        for oc in range(NOCT):
            e8h_sb = p_e8.tile([128, 2, 2, OCT, NE], f8, tag="e8h")
            nc.sync.dma_start(out=e8h_sb, in_=e8h_d[oc, :, :, :, :, :])
            e8l_sb = p_e8.tile([128, 2, 2, OCT, NE], f8, tag="e8l")
            nc.sync.dma_start(out=e8l_sb, in_=e8l_d[oc, :, :, :, :, :])
            e8s_sb = p_e8.tile([128, 2, 2, OCT, NE], f8, tag="e8s")
            nc.scalar.dma_start(out=e8s_sb, in_=e8s_d[oc, :, :, :, :, :])
            q8h_sb = p_e8.tile([128, 2, 2, OCT, NQ], f8, tag="q8h")
            nc.gpsimd.dma_start(out=q8h_sb, in_=q8h_d[oc, :, :, :, :, :])
            q8l_sb = p_e8.tile([128, 2, 2, OCT, NQ], f8, tag="q8l")
            nc.gpsimd.dma_start(out=q8l_sb, in_=q8l_d[oc, :, :, :, :, :])
            q8s_sb = p_e8.tile([128, 2, 2, OCT, NQ], f8, tag="q8s")
            nc.gpsimd.dma_start(out=q8s_sb, in_=q8s_d[oc, :, :, :, :, :])
            keep_sb = p_keep.tile([128, OCT, 2, NQ], bf16, tag="keep")
            nc.sync.dma_start(out=keep_sb, in_=keep_d[oc, :, :, :, :])

            # ---- Q^T for the whole octet: amortize W_q weight loads ----
            qz = qz_bufs[oc % 2]
            for m in range(4):
                ps_q = pp.tile([128, OCT * HD], f32, tag="proj", name="ps_q", bufs=3)
                idx = 0
                for wt, et in ((w8h_sb, q8h_sb), (w8h_sb, q8l_sb),
                               (w8ls_sb, q8s_sb)):
                    for j in range(2):
                        nc.tensor.matmul(
                            ps_q,
                            lhsT=wt[:, j, :, m * 128:(m + 1) * 128],
                            rhs=et[:, j, :, :, :],
                            start=(idx == 0),
                            stop=(idx == 5),
                            perf_mode=DR,
                        )
                        idx += 1
                nc.scalar.copy(out=qz[0:64, m, 0, :, :], in_=ps_q[0:64, :])
                nc.scalar.copy(out=qz[64:128, m, 1, :, :], in_=ps_q[64:128, :])

            for pr in range(PAIRS_PER_OCT):
                lb = pr * 2          # local batch index in octet
                gpair = oc * PAIRS_PER_OCT + pr

                # ---- K^T ----
                kT_sb = p_kT.tile([128, 4, 2, NE], bf16, tag="kT")
                for m in range(4):
                    ps_k = pp.tile([128, 2 * NE], f32, tag="proj", name="ps_k", bufs=3)
                    idx = 0
                    for wt, et in ((w8h_sb, e8h_sb), (w8h_sb, e8l_sb),
                                   (w8ls_sb, e8s_sb)):
                        for j in range(2):
                            nc.tensor.matmul(
                                ps_k,
                                lhsT=wt[:, j, :, ED + m * 128:ED + (m + 1) * 128],
                                rhs=et[:, j, :, lb:lb + 2, :],
                                start=(idx == 0),
                                stop=(idx == 5),
                                perf_mode=DR,
                            )
                            idx += 1
                    if m % 2 == 0:
                        nc.vector.tensor_copy(out=kT_sb[:, m, :, :], in_=ps_k)
                    else:
                        nc.scalar.copy(out=kT_sb[:, m, :, :], in_=ps_k)

                # ---- V ----
                v_sb = p_v.tile([128, 2, 2, ED], bf16, tag="v")
                for n2 in range(2):
                    for b2 in range(2):
                        ps_v = pp.tile([128, ED], f32, tag="proj", name="ps_v", bufs=3)
                        idx = 0
                        for et, wt in ((e8h_sb, w8h_sb), (e8l_sb, w8h_sb),
                                       (e8s_sb, w8ls_sb)):
                            for j in range(2):
                                nc.tensor.matmul(
                                    ps_v,
                                    lhsT=et[:, j, :, lb + b2, n2 * 128:(n2 + 1) * 128],
                                    rhs=wt[:, j, :, 2 * ED:3 * ED],
                                    start=(idx == 0),
                                    stop=(idx == 5),
                                    perf_mode=DR,
                                )
                                idx += 1
                        if b2 == 0:
                            nc.scalar.copy(out=v_sb[:, n2, b2, :], in_=ps_v)
                        else:
                            nc.vector.tensor_copy(out=v_sb[:, n2, b2, :], in_=ps_v)

                # ---- logits^T + exp + keep-mask ----
                # wm[(b2, n2)] : [128(ne-slice), H*NQ] bf16
                wm = {}
                for n2 in range(2):
                    for b2 in range(2):
                        ps_l = pp.tile([128, H * NQ], f32, tag="logit", name="ps_l", bufs=2)
                        for h in range(H):
                            nc.tensor.matmul(
                                ps_l[:, h * NQ:(h + 1) * NQ],
                                lhsT=kT_sb[:, h // 2, b2,
                                           n2 * 128:(n2 + 1) * 128],
                                rhs=qz[:, h // 2, h % 2, lb + b2, :],
                                start=True,
                                stop=True,
                            )
                        wm_t = p_wm.tile([128, H * NQ], bf16, tag="wm", name="wm_t")
                        nc.scalar.activation(
                            out=wm_t, in_=ps_l,
                            func=mybir.ActivationFunctionType.Exp,
                            scale=SCALE,
                        )
                        keep_rep = keep_sb[:, lb + b2, n2, None, :].broadcast_to(
                            [128, H, NQ])
                        nc.gpsimd.tensor_mul(wm_t, wm_t, keep_rep)
                        wm[(b2, n2)] = wm_t

                # ---- softmax denominators (PE broadcast) + attn ----
                recip = {}
                for b2 in range(2):
                    wmsum = p_wmsum.tile([128, H * NQ], bf16, tag="wmsum")
                    nc.gpsimd.tensor_add(wmsum, wm[(b2, 0)], wm[(b2, 1)])
                    ps_s = pp.tile([128, H * NQ], f32, tag="sums", name="ps_s", bufs=1)
                    nc.tensor.matmul(
                        ps_s,
                        lhsT=ones_sb,
                        rhs=wmsum,
                        start=True,
                        stop=True,
                    )
                    r_sb = p_recip.tile([128, H * NQ], f32, tag="recip", name="r_sb")
                    nc.vector.reciprocal_approx_fast(out=r_sb, in_=ps_s)
                    recip[b2] = r_sb

                ps_a = pp.tile([128, 512], f32, tag="attn", name="ps_a")
                for hp in range(4):
                    for b2 in range(2):
                        for h2 in range(2):
                            h = 2 * hp + h2
                            col = (hp * 2 + b2) * 64
                            for n2 in range(2):
                                nc.tensor.matmul(
                                    ps_a[h2 * 64:(h2 + 1) * 64, col:col + 64],
                                    lhsT=v_sb[:, n2, b2, h * 64:(h + 1) * 64],
                                    rhs=wm[(b2, n2)][:, h * 64:(h + 1) * 64],
                                    start=(n2 == 0),
                                    stop=(n2 == 1),
                                )

                # normalize -> attn_sb (bf16), layout [128(2-head rows), (hp, b2, q)]
                attn_sb = p_attn.tile([128, 512], bf16, tag="attn_sb")
                for b2 in range(2):
                    for h2 in range(2):
                        rows = slice(h2 * 64, (h2 + 1) * 64)
                        o_ap = attn_sb[rows, :].rearrange(
                            "p (hp b q) -> p hp b q", hp=4, b=2)[:, :, b2, :]
                        i_ap = ps_a[rows, :].rearrange(
                            "p (hp b q) -> p hp b q", hp=4, b=2)[:, :, b2, :]
                        r_ap = recip[b2][rows, :].rearrange(
                            "p (hp x) -> p hp x", hp=4)[:, :, h2 * 64:(h2 + 1) * 64]
                        nc.vector.tensor_mul(o_ap, i_ap, r_ap)

                # ---- output projection + bias + post mask ----
                ps_o = pp.tile([128, OUT_DIM], f32, tag="attn", name="ps_o")
                for t in range(4):
                    nc.tensor.matmul(
                        ps_o,
                        lhsT=attn_sb[:, t * 128:(t + 1) * 128],
                        rhs=wo_sb[:, t, :],
                        start=(t == 0),
                        stop=(t == 3),
                    )
                out_sb = p_out.tile([128, OUT_DIM], f32, tag="out_sb")
                nc.vector.tensor_scalar_mul(
                    out_sb, in0=ps_o, scalar1=postm_sb[:, gpair:gpair + 1])
                nc.vector.scalar_tensor_tensor(
                    out_sb, in0=bias_bc,
                    scalar=postm_sb[:, gpair:gpair + 1],
                    in1=out_sb,
                    op0=mybir.AluOpType.mult, op1=mybir.AluOpType.add)
                nc.sync.dma_start(
                    out=out_d[gpair * 2:gpair * 2 + 2].rearrange("b q d -> (b q) d"),
                    in_=out_sb,
                )
    nc.compile()
    return nc


def _prep_inputs(entities, pre_mask, post_mask, W_in, W_out, b_out):
    """Host-side sharding + layout transforms (not part of timed HW work)."""
    f32 = np.float32
    # entities -> E^T fp8 hi/lo/shifted, DoubleRow plane layout:
    # [g, p, j, t, b, ne] where in_dim = (2j + t)*128 + p
    x = entities.reshape(BS // OCT, OCT, NE, 2, 2, 128).transpose(0, 5, 3, 4, 1, 2)
    x = np.ascontiguousarray(x, dtype=f32)
    e8h = x.astype(F8)
    e8l = (x - e8h.astype(f32)).astype(F8)
    e8s = (e8h.astype(f32) / LO_S).astype(F8)
    # queries-only variant (first NQ entities)
    xq = np.ascontiguousarray(x[..., :NQ])
    q8h = xq.astype(F8)
    q8l = (xq - q8h.astype(f32)).astype(F8)
    q8s = (q8h.astype(f32) / LO_S).astype(F8)

    keep = (1 - pre_mask).astype(f32).reshape(
        BS // OCT, OCT, NQ, 2, 128).transpose(0, 4, 1, 3, 2).astype(BF16)
    keep = np.ascontiguousarray(keep)
    kpost = (1 - post_mask).astype(f32)  # [BS, NQ]

    w4 = np.ascontiguousarray(W_in, dtype=f32).reshape(2, 2, 128, 3 * ED)
    w4 = np.ascontiguousarray(w4.transpose(2, 0, 1, 3))  # [p, j, t, col]
    w8h = w4.astype(F8)
    w8ls = ((w4 - w8h.astype(f32)) * LO_S).astype(F8)
    w_out = np.ascontiguousarray(W_out).reshape(4, 128, OUT_DIM).astype(BF16)
    b_o = b_out.reshape(1, OUT_DIM).astype(f32)

    in_maps = []
    for c in range(NCORES):
        sl = slice(c * BPC, (c + 1) * BPC)
        # postm: [128 rows=(b2*64+q), NPAIRS]
        kp = kpost[sl].reshape(NPAIRS, 128).T.copy()
        g = slice(c * NOCT, (c + 1) * NOCT)
        in_maps.append({
            "e8h": np.ascontiguousarray(e8h[g]),
            "e8l": np.ascontiguousarray(e8l[g]),
            "e8s": np.ascontiguousarray(e8s[g]),
            "q8h": np.ascontiguousarray(q8h[g]),
            "q8l": np.ascontiguousarray(q8l[g]),
            "q8s": np.ascontiguousarray(q8s[g]),
            "keep": np.ascontiguousarray(keep[g]),
            "postm": np.ascontiguousarray(kp),
            "w8h": w8h,
            "w8ls": w8ls,
            "w_out": w_out,
            "b_out": b_o,
        })
    return in_maps


def kernel(entities, pre_mask, post_mask, W_in, W_out, b_out, trace=False):
    global LAST_RESULT
    from concourse.bass_utils import run_bass_kernel_spmd

    if "nc" not in _BUILT:
        _BUILT["nc"] = _build_nc()
    nc = _BUILT["nc"]

    in_maps = _prep_inputs(np.asarray(entities), np.asarray(pre_mask),
                           np.asarray(post_mask), np.asarray(W_in),
                           np.asarray(W_out), np.asarray(b_out))
    res = run_bass_kernel_spmd(nc, in_maps, core_ids=list(range(NCORES)),
                               trace=trace)
    LAST_RESULT = res
    out = np.concatenate([r["out"] for r in res.results], axis=0)
    return out.astype(np.float32)


# revision 5
# speedup vs baseline: 1.3222x; 1.3222x over previous
"""Trainium2 Bass kernel for nn_EntityAttentionLayer (sparse attention).

Strategy (8 cores, data-parallel over bs):
  - Host side: shard bs across 8 cores (64 items each), pre-transpose
    entities to E^T[in_dim, ne] per batch, cast operands to bf16,
    convert masks to multiplicative keep-masks.
  - On chip, per batch b (processed in pairs, Q in octets of 8):
      K^T[ed, ne]  = (Wk^T E^T)        via lhsT=Wk slices, rhs=E^T
      V[ne, ed]    = E V-proj          via lhsT=E^T slices, rhs=Wv
      Q^T[ed, q]   =                   via lhsT=Wq slices, rhs=E^T[:, :64]
      logits^T[ne, q] per head         lhsT=K^T_h, rhs=Q^T_h  (ne on partitions)
      wm = exp(logits * 1/sqrt(hd))    on ACT (scale folded into activation)
      wm *= keep^T                     on DVE (multiplicative mask, h-broadcast)
      wmsum = wm(n2=0)+wm(n2=1)        on GPSIMD (halves the PE sums matmuls)
      sums broadcast [128, h*q]        single PE matmul with all-ones lhsT
      attn^T unnorm [2-heads, hp*b*q]  lhsT=V slices, rhs=wm  (col-tiled pairs)
      attn = attn_unnorm * 1/sums      DVE (approx reciprocal + strided muls)
      out[b*q, out] = attn^T.T @ W_out (+bias & post-mask fused on DVE)
      out *= keep_post (per-partition scalar), DMA out.
  All matmuls bf16 operands, fp32 PSUM accumulation.
"""

import numpy as np
import ml_dtypes

BS, NE, NQ, IN_DIM, ED, OUT_DIM, H, HD = 512, 256, 64, 512, 512, 512, 8, 64
NCORES = 8
BPC = BS // NCORES          # 64 batches per core
OCT = 8                     # batches per super-batch (Q^T amortization)
NOCT = BPC // OCT           # 8
PAIRS_PER_OCT = OCT // 2    # 4
NPAIRS = BPC // 2           # 32
SCALE = 1.0 / float(np.sqrt(HD))

BF16 = ml_dtypes.bfloat16

_BUILT = {}
LAST_RESULT = None


def _build_nc():
    import concourse.tile as tile
    from concourse import bacc, mybir
    from contextlib import ExitStack

    f32 = mybir.dt.float32
    bf16 = mybir.dt.bfloat16

    nc = bacc.Bacc("TRN2", target_bir_lowering=False)

    eT_d = nc.dram_tensor("eT", [NOCT, 128, OCT, 4, NE], bf16, kind="ExternalInput")
    keep_d = nc.dram_tensor("keep", [NOCT, 128, OCT, 2, NQ], bf16, kind="ExternalInput")
    postm_d = nc.dram_tensor("postm", [128, NPAIRS], f32, kind="ExternalInput")
    w_in_d = nc.dram_tensor("w_in", [4, 128, 3 * ED], bf16, kind="ExternalInput")
    w_out_d = nc.dram_tensor("w_out", [4, 128, OUT_DIM], bf16, kind="ExternalInput")
    b_out_d = nc.dram_tensor("b_out", [1, OUT_DIM], f32, kind="ExternalInput")
    out_d = nc.dram_tensor("out", [BPC, NQ, OUT_DIM], f32, kind="ExternalOutput")

    with ExitStack() as ctx:
        tc = ctx.enter_context(tile.TileContext(nc))
        consts = ctx.enter_context(tc.tile_pool(name="consts", bufs=1))
        p_eT = ctx.enter_context(tc.tile_pool(name="p_eT", bufs=3))
        p_keep = ctx.enter_context(tc.tile_pool(name="p_keep", bufs=3))
        p_kT = ctx.enter_context(tc.tile_pool(name="p_kT", bufs=3))
        p_v = ctx.enter_context(tc.tile_pool(name="p_v", bufs=3))
        p_wm = ctx.enter_context(tc.tile_pool(name="p_wm", bufs=6))
        p_wmsum = ctx.enter_context(tc.tile_pool(name="p_wmsum", bufs=3))
        p_recip = ctx.enter_context(tc.tile_pool(name="p_recip", bufs=2))
        p_attn = ctx.enter_context(tc.tile_pool(name="p_attn", bufs=2))
        p_out = ctx.enter_context(tc.tile_pool(name="p_out", bufs=3))
        pp = ctx.enter_context(tc.tile_pool(name="pp", bufs=2, space="PSUM"))

        # Constants
        w_sb = consts.tile([128, 4, 3 * ED], bf16)
        for wc, eng in ((0, nc.scalar), (1, nc.gpsimd), (2, nc.scalar)):
            eng.dma_start(
                out=w_sb[:, :, wc * ED:(wc + 1) * ED],
                in_=w_in_d[:, :, wc * ED:(wc + 1) * ED].rearrange("k p n -> p k n"))
        wo_sb = consts.tile([128, 4, OUT_DIM], bf16)
        nc.gpsimd.dma_start(out=wo_sb, in_=w_out_d[:, :, :].rearrange("k p n -> p k n"))
        bias_bc = consts.tile([128, OUT_DIM], f32)
        nc.gpsimd.dma_start(out=bias_bc, in_=b_out_d[:, :].to_broadcast([128, OUT_DIM]))
        postm_sb = consts.tile([128, NPAIRS], f32)
        nc.gpsimd.dma_start(out=postm_sb, in_=postm_d[:, :])
        ones_sb = consts.tile([128, 128], bf16)
        nc.vector.memset(ones_sb, 1.0)
        # Persistent zero-padded Q^T tiles (manual double buffer by octet
        # parity). Layout [128, m, h2, b, q]: head parity h2 selects which
        # 64-row half holds data; the other half stays zero so logits
        # matmuls can use full K=128 operands at base partition 0
        # (operands at base partition 64 fault on HW).
        qz0 = consts.tile([128, 4, 2, OCT, HD], bf16)
        nc.vector.memset(qz0, 0.0)
        qz1 = consts.tile([128, 4, 2, OCT, HD], bf16)
        nc.vector.memset(qz1, 0.0)
        qz_bufs = [qz0, qz1]

        for oc in range(NOCT):
            # Pair-aligned DMA chunks: the K-proj of pair p depends only on
            # its own 2-batch slice, so compute can start on chunk 0.
            eT_sb = p_eT.tile([128, OCT, 4, NE], bf16, tag="eT")
            for pc, eng in ((0, nc.sync), (1, nc.scalar), (2, nc.sync),
                            (3, nc.scalar)):
                eng.dma_start(
                    out=eT_sb[:, pc * 2:(pc + 1) * 2, :, :],
                    in_=eT_d[oc, :, pc * 2:(pc + 1) * 2, :, :],
                )
            keep_sb = p_keep.tile([128, OCT, 2, NQ], bf16, tag="keep")
            nc.sync.dma_start(out=keep_sb, in_=keep_d[oc, :, :, :, :])

            # ---- Q^T for the whole octet: amortize W_q weight loads ----
            qz = qz_bufs[oc % 2]
            for m in range(4):
                ps_q = pp.tile([128, OCT * HD], f32, tag="proj", name="ps_q", bufs=3)
                for bc in (0, 4):
                    for k in range(4):
                        nc.tensor.matmul(
                            ps_q[:, bc * HD:(bc + 4) * HD],
                            lhsT=w_sb[:, k, m * 128:(m + 1) * 128],
                            rhs=eT_sb[:, bc:bc + 4, k, 0:NQ],
                            start=(k == 0),
                            stop=(k == 3),
                        )
                nc.scalar.copy(out=qz[0:64, m, 0, :, :], in_=ps_q[0:64, :])
                nc.scalar.copy(out=qz[64:128, m, 1, :, :], in_=ps_q[64:128, :])

            for pr in range(PAIRS_PER_OCT):
                lb = pr * 2          # local batch index in octet
                gpair = oc * PAIRS_PER_OCT + pr

                # ---- K^T ----
                kT_sb = p_kT.tile([128, 4, 2, NE], bf16, tag="kT")
                for m in range(4):
                    ps_k = pp.tile([128, 2 * NE], f32, tag="proj", name="ps_k", bufs=3)
                    for k in range(4):
                        nc.tensor.matmul(
                            ps_k,
                            lhsT=w_sb[:, k, ED + m * 128:ED + (m + 1) * 128],
                            rhs=eT_sb[:, lb:lb + 2, k, :],
                            start=(k == 0),
                            stop=(k == 3),
                        )
                    if m % 2 == 0:
                        nc.vector.tensor_copy(out=kT_sb[:, m, :, :], in_=ps_k)
                    else:
                        nc.scalar.copy(out=kT_sb[:, m, :, :], in_=ps_k)

                # ---- V ----
                v_sb = p_v.tile([128, 2, 2, ED], bf16, tag="v")
                for n2 in range(2):
                    for b2 in range(2):
                        ps_v = pp.tile([128, ED], f32, tag="proj", name="ps_v", bufs=3)
                        for k in range(4):
                            nc.tensor.matmul(
                                ps_v,
                                lhsT=eT_sb[:, lb + b2, k, n2 * 128:(n2 + 1) * 128],
                                rhs=w_sb[:, k, 2 * ED:3 * ED],
                                start=(k == 0),
                                stop=(k == 3),
                            )
                        if b2 == 0:
                            nc.scalar.copy(out=v_sb[:, n2, b2, :], in_=ps_v)
                        else:
                            nc.vector.tensor_copy(out=v_sb[:, n2, b2, :], in_=ps_v)

                # ---- logits^T + exp + keep-mask ----
                # wm[(b2, n2)] : [128(ne-slice), H*NQ] bf16
                wm = {}
                for n2 in range(2):
                    for b2 in range(2):
                        ps_l = pp.tile([128, H * NQ], f32, tag="logit", name="ps_l", bufs=3)
                        for h in range(H):
                            nc.tensor.matmul(
                                ps_l[:, h * NQ:(h + 1) * NQ],
                                lhsT=kT_sb[:, h // 2, b2,
                                           n2 * 128:(n2 + 1) * 128],
                                rhs=qz[:, h // 2, h % 2, lb + b2, :],
                                start=True,
                                stop=True,
                            )
                        wm_t = p_wm.tile([128, H * NQ], bf16, tag="wm", name="wm_t")
                        nc.scalar.activation(
                            out=wm_t, in_=ps_l,
                            func=mybir.ActivationFunctionType.Exp,
                            scale=SCALE,
                        )
                        keep_rep = keep_sb[:, lb + b2, n2, None, :].broadcast_to(
                            [128, H, NQ])
                        nc.vector.tensor_mul(wm_t, wm_t, keep_rep)
                        wm[(b2, n2)] = wm_t

                # ---- softmax denominators (PE broadcast) + attn ----
                recip = {}
                for b2 in range(2):
                    wmsum = p_wmsum.tile([128, H * NQ], bf16, tag="wmsum")
                    nc.gpsimd.tensor_add(wmsum, wm[(b2, 0)], wm[(b2, 1)])
                    ps_s = pp.tile([128, H * NQ], f32, tag="logit", name="ps_s", bufs=3)
                    nc.tensor.matmul(
                        ps_s,
                        lhsT=ones_sb,
                        rhs=wmsum,
                        start=True,
                        stop=True,
                    )
                    r_sb = p_recip.tile([128, H * NQ], f32, tag="recip", name="r_sb")
                    nc.vector.reciprocal_approx_fast(out=r_sb, in_=ps_s)
                    recip[b2] = r_sb

                ps_a = pp.tile([128, 512], f32, tag="attn", name="ps_a")
                for hp in range(4):
                    for b2 in range(2):
                        for h2 in range(2):
                            h = 2 * hp + h2
                            col = (hp * 2 + b2) * 64
                            for n2 in range(2):
                                nc.tensor.matmul(
                                    ps_a[h2 * 64:(h2 + 1) * 64, col:col + 64],
                                    lhsT=v_sb[:, n2, b2, h * 64:(h + 1) * 64],
                                    rhs=wm[(b2, n2)][:, h * 64:(h + 1) * 64],
                                    start=(n2 == 0),
                                    stop=(n2 == 1),
                                )

                # normalize -> attn_sb (bf16), layout [128(2-head rows), (hp, b2, q)]
                attn_sb = p_attn.tile([128, 512], bf16, tag="attn_sb")
                for b2 in range(2):
                    for h2 in range(2):
                        rows = slice(h2 * 64, (h2 + 1) * 64)
                        o_ap = attn_sb[rows, :].rearrange(
                            "p (hp b q) -> p hp b q", hp=4, b=2)[:, :, b2, :]
                        i_ap = ps_a[rows, :].rearrange(
                            "p (hp b q) -> p hp b q", hp=4, b=2)[:, :, b2, :]
                        r_ap = recip[b2][rows, :].rearrange(
                            "p (hp x) -> p hp x", hp=4)[:, :, h2 * 64:(h2 + 1) * 64]
                        nc.vector.tensor_mul(o_ap, i_ap, r_ap)

                # ---- output projection + bias + post mask ----
                ps_o = pp.tile([128, OUT_DIM], f32, tag="attn", name="ps_o")
                for t in range(4):
                    nc.tensor.matmul(
                        ps_o,
                        lhsT=attn_sb[:, t * 128:(t + 1) * 128],
                        rhs=wo_sb[:, t, :],
                        start=(t == 0),
                        stop=(t == 3),
                    )
                out_sb = p_out.tile([128, OUT_DIM], f32, tag="out_sb")
                nc.vector.tensor_scalar_mul(
                    out_sb, in0=ps_o, scalar1=postm_sb[:, gpair:gpair + 1])
                nc.vector.scalar_tensor_tensor(
                    out_sb, in0=bias_bc,
                    scalar=postm_sb[:, gpair:gpair + 1],
                    in1=out_sb,
                    op0=mybir.AluOpType.mult, op1=mybir.AluOpType.add)
                nc.sync.dma_start(
                    out=out_d[gpair * 2:gpair * 2 + 2].rearrange("b q d -> (b q) d"),
                    in_=out_sb,
                )
    nc.compile()
    return nc


def _prep_inputs(entities, pre_mask, post_mask, W_in, W_out, b_out):
    """Host-side sharding + layout transforms (not part of timed HW work)."""
    # [oct, 128p, b, k, ne] contiguous per partition row
    eT = entities.reshape(BS // OCT, OCT, NE, 4, 128).transpose(
        0, 4, 1, 3, 2).astype(BF16)
    eT = np.ascontiguousarray(eT)
    keep = (1 - pre_mask).astype(np.float32).reshape(
        BS // OCT, OCT, NQ, 2, 128).transpose(0, 4, 1, 3, 2).astype(BF16)
    keep = np.ascontiguousarray(keep)
    kpost = (1 - post_mask).astype(np.float32)  # [BS, NQ]
    w_in = np.ascontiguousarray(W_in).reshape(4, 128, 3 * ED).astype(BF16)
    w_out = np.ascontiguousarray(W_out).reshape(4, 128, OUT_DIM).astype(BF16)
    b_o = b_out.reshape(1, OUT_DIM).astype(np.float32)

    in_maps = []
    for c in range(NCORES):
        sl = slice(c * BPC, (c + 1) * BPC)
        # postm: [128 rows=(b2*64+q), NPAIRS]
        kp = kpost[sl].reshape(NPAIRS, 128).T.copy()
        in_maps.append({
            "eT": np.ascontiguousarray(eT[c * NOCT:(c + 1) * NOCT]),
            "keep": np.ascontiguousarray(keep[c * NOCT:(c + 1) * NOCT]),
            "postm": np.ascontiguousarray(kp),
            "w_in": w_in,
            "w_out": w_out,
            "b_out": b_o,
        })
    return in_maps


def kernel(entities, pre_mask, post_mask, W_in, W_out, b_out, trace=False):
    global LAST_RESULT
    from concourse.bass_utils import run_bass_kernel_spmd

    if "nc" not in _BUILT:
        _BUILT["nc"] = _build_nc()
    nc = _BUILT["nc"]

    in_maps = _prep_inputs(np.asarray(entities), np.asarray(pre_mask),
                           np.asarray(post_mask), np.asarray(W_in),
                           np.asarray(W_out), np.asarray(b_out))
    res = run_bass_kernel_spmd(nc, in_maps, core_ids=list(range(NCORES)),
                               trace=trace)
    LAST_RESULT = res
    out = np.concatenate([r["out"] for r in res.results], axis=0)
    return out.astype(np.float32)


# revision 8
# speedup vs baseline: 1.3959x; 1.0558x over previous
"""Trainium2 Bass kernel for nn_EntityAttentionLayer (sparse attention).

Strategy (8 cores, data-parallel over bs):
  - Host side: shard bs across 8 cores (64 items each), pre-transpose
    entities to E^T[in_dim, ne] per batch, cast operands to bf16,
    convert masks to multiplicative keep-masks.
  - On chip, per batch b (processed in pairs, Q in octets of 8):
      K^T[ed, ne]  = (Wk^T E^T)        via lhsT=Wk slices, rhs=E^T
      V[ne, ed]    = E V-proj          via lhsT=E^T slices, rhs=Wv
      Q^T[ed, q]   =                   via lhsT=Wq slices, rhs=E^T[:, :64]
      logits^T[ne, q] per head         lhsT=K^T_h, rhs=Q^T_h  (ne on partitions)
      wm = exp(logits * 1/sqrt(hd))    on ACT (scale folded into activation)
      wm *= keep^T                     on DVE (multiplicative mask, h-broadcast)
      wmsum = wm(n2=0)+wm(n2=1)        on GPSIMD (halves the PE sums matmuls)
      sums broadcast [128, h*q]        single PE matmul with all-ones lhsT
      attn^T unnorm [2-heads, hp*b*q]  lhsT=V slices, rhs=wm  (col-tiled pairs)
      attn = attn_unnorm * 1/sums      DVE (approx reciprocal + strided muls)
      out[b*q, out] = attn^T.T @ W_out (+bias & post-mask fused on DVE)
      out *= keep_post (per-partition scalar), DMA out.
  All matmuls bf16 operands, fp32 PSUM accumulation.
"""

import numpy as np
import ml_dtypes

BS, NE, NQ, IN_DIM, ED, OUT_DIM, H, HD = 512, 256, 64, 512, 512, 512, 8, 64
NCORES = 8
BPC = BS // NCORES          # 64 batches per core
OCT = 8                     # batches per super-batch (Q^T amortization)
NOCT = BPC // OCT           # 8
PAIRS_PER_OCT = OCT // 2    # 4
NPAIRS = BPC // 2           # 32
SCALE = 1.0 / float(np.sqrt(HD))

BF16 = ml_dtypes.bfloat16

_BUILT = {}
LAST_RESULT = None


def _build_nc():
    import concourse.tile as tile
    from concourse import bacc, mybir
    from contextlib import ExitStack

    f32 = mybir.dt.float32
    bf16 = mybir.dt.bfloat16

    nc = bacc.Bacc("TRN2", target_bir_lowering=False)

    eT_d = nc.dram_tensor("eT", [NOCT, 128, OCT, 4, NE], bf16, kind="ExternalInput")
    keep_d = nc.dram_tensor("keep", [NOCT, 128, OCT, 2, NQ], bf16, kind="ExternalInput")
    postm_d = nc.dram_tensor("postm", [128, NPAIRS], f32, kind="ExternalInput")
    w_in_d = nc.dram_tensor("w_in", [4, 128, 3 * ED], bf16, kind="ExternalInput")
    w_out_d = nc.dram_tensor("w_out", [4, 128, OUT_DIM], bf16, kind="ExternalInput")
    b_out_d = nc.dram_tensor("b_out", [1, OUT_DIM], f32, kind="ExternalInput")
    out_d = nc.dram_tensor("out", [BPC, NQ, OUT_DIM], f32, kind="ExternalOutput")

    with ExitStack() as ctx:
        tc = ctx.enter_context(tile.TileContext(nc))
        consts = ctx.enter_context(tc.tile_pool(name="consts", bufs=1))
        p_eT = ctx.enter_context(tc.tile_pool(name="p_eT", bufs=3))
        p_keep = ctx.enter_context(tc.tile_pool(name="p_keep", bufs=3))
        p_kT = ctx.enter_context(tc.tile_pool(name="p_kT", bufs=3))
        p_v = ctx.enter_context(tc.tile_pool(name="p_v", bufs=3))
        p_wm = ctx.enter_context(tc.tile_pool(name="p_wm", bufs=6))
        p_wmsum = ctx.enter_context(tc.tile_pool(name="p_wmsum", bufs=3))
        p_recip = ctx.enter_context(tc.tile_pool(name="p_recip", bufs=2))
        p_attn = ctx.enter_context(tc.tile_pool(name="p_attn", bufs=2))
        p_out = ctx.enter_context(tc.tile_pool(name="p_out", bufs=3))
        pp = ctx.enter_context(tc.tile_pool(name="pp", bufs=2, space="PSUM"))

        # Constants
        w_sb = consts.tile([128, 4, 3 * ED], bf16)
        for wc, eng in ((0, nc.scalar), (1, nc.gpsimd), (2, nc.scalar)):
            eng.dma_start(
                out=w_sb[:, :, wc * ED:(wc + 1) * ED],
                in_=w_in_d[:, :, wc * ED:(wc + 1) * ED].rearrange("k p n -> p k n"))
        wo_sb = consts.tile([128, 4, OUT_DIM], bf16)
        nc.gpsimd.dma_start(out=wo_sb, in_=w_out_d[:, :, :].rearrange("k p n -> p k n"))
        bias_bc = consts.tile([128, OUT_DIM], f32)
        nc.gpsimd.dma_start(out=bias_bc, in_=b_out_d[:, :].to_broadcast([128, OUT_DIM]))
        postm_sb = consts.tile([128, NPAIRS], f32)
        nc.gpsimd.dma_start(out=postm_sb, in_=postm_d[:, :])
        ones_sb = consts.tile([128, 128], bf16)
        nc.vector.memset(ones_sb, 1.0)
        # Persistent zero-padded Q^T tiles (manual double buffer by octet
        # parity). Layout [128, m, h2, b, q]: head parity h2 selects which
        # 64-row half holds data; the other half stays zero so logits
        # matmuls can use full K=128 operands at base partition 0
        # (operands at base partition 64 fault on HW).
        qz0 = consts.tile([128, 4, 2, OCT, HD], bf16)
        nc.vector.memset(qz0, 0.0)
        qz1 = consts.tile([128, 4, 2, OCT, HD], bf16)
        nc.vector.memset(qz1, 0.0)
        qz_bufs = [qz0, qz1]

        for oc in range(NOCT):
            # Pair-aligned DMA chunks: the K-proj of pair p depends only on
            # its own 2-batch slice, so compute can start on chunk 0.
            eT_sb = p_eT.tile([128, OCT, 4, NE], bf16, tag="eT")
            for pc, eng in ((0, nc.sync), (1, nc.scalar), (2, nc.sync),
                            (3, nc.scalar)):
                eng.dma_start(
                    out=eT_sb[:, pc * 2:(pc + 1) * 2, :, :],
                    in_=eT_d[oc, :, pc * 2:(pc + 1) * 2, :, :],
                )
            keep_sb = p_keep.tile([128, OCT, 2, NQ], bf16, tag="keep")
            nc.sync.dma_start(out=keep_sb, in_=keep_d[oc, :, :, :, :])

            # ---- Q^T for the whole octet: amortize W_q weight loads ----
            qz = qz_bufs[oc % 2]
            for m in range(4):
                ps_q = pp.tile([128, OCT * HD], f32, tag="proj", name="ps_q", bufs=3)
                for k in range(4):
                    nc.tensor.matmul(
                        ps_q,
                        lhsT=w_sb[:, k, m * 128:(m + 1) * 128],
                        rhs=eT_sb[:, :, k, 0:NQ],
                        start=(k == 0),
                        stop=(k == 3),
                    )
                nc.scalar.copy(out=qz[0:64, m, 0, :, :], in_=ps_q[0:64, :])
                nc.scalar.copy(out=qz[64:128, m, 1, :, :], in_=ps_q[64:128, :])

            for pr in range(PAIRS_PER_OCT):
                lb = pr * 2          # local batch index in octet
                gpair = oc * PAIRS_PER_OCT + pr

                # ---- K^T ----
                kT_sb = p_kT.tile([128, 4, 2, NE], bf16, tag="kT")
                for m in range(4):
                    ps_k = pp.tile([128, 2 * NE], f32, tag="proj", name="ps_k", bufs=3)
                    for k in range(4):
                        nc.tensor.matmul(
                            ps_k,
                            lhsT=w_sb[:, k, ED + m * 128:ED + (m + 1) * 128],
                            rhs=eT_sb[:, lb:lb + 2, k, :],
                            start=(k == 0),
                            stop=(k == 3),
                        )
                    if m % 2 == 0:
                        nc.vector.tensor_copy(out=kT_sb[:, m, :, :], in_=ps_k)
                    else:
                        nc.scalar.copy(out=kT_sb[:, m, :, :], in_=ps_k)

                # ---- V ----
                v_sb = p_v.tile([128, 2, 2, ED], bf16, tag="v")
                for n2 in range(2):
                    for b2 in range(2):
                        ps_v = pp.tile([128, ED], f32, tag="proj", name="ps_v", bufs=3)
                        for k in range(4):
                            nc.tensor.matmul(
                                ps_v,
                                lhsT=eT_sb[:, lb + b2, k, n2 * 128:(n2 + 1) * 128],
                                rhs=w_sb[:, k, 2 * ED:3 * ED],
                                start=(k == 0),
                                stop=(k == 3),
                            )
                        nc.scalar.copy(out=v_sb[:, n2, b2, :], in_=ps_v)

                # ---- logits^T + exp + keep-mask ----
                # wm[(b2, n2)] : [128(ne-slice), H*NQ] bf16
                wm = {}
                for n2 in range(2):
                    for b2 in range(2):
                        ps_l = pp.tile([128, H * NQ], f32, tag="logit", name="ps_l", bufs=2)
                        for hp in range(4):
                            # head pair shares the same K^T stationary slice;
                            # qz head-halves are complementarily zero-padded
                            nc.tensor.matmul(
                                ps_l[:, hp * 2 * NQ:(hp + 1) * 2 * NQ],
                                lhsT=kT_sb[:, hp, b2,
                                           n2 * 128:(n2 + 1) * 128],
                                rhs=qz[:, hp, :, lb + b2, :],
                                start=True,
                                stop=True,
                            )
                        wm_t = p_wm.tile([128, H * NQ], bf16, tag="wm", name="wm_t")
                        nc.scalar.activation(
                            out=wm_t, in_=ps_l,
                            func=mybir.ActivationFunctionType.Exp,
                            scale=SCALE,
                        )
                        keep_rep = keep_sb[:, lb + b2, n2, None, :].broadcast_to(
                            [128, H, NQ])
                        # n2=0 masks on the idle GPSIMD; n2=1 on DVE so the
                        # wmsum add right after it runs in-order (no hop)
                        if n2 == 0:
                            nc.gpsimd.tensor_mul(wm_t, wm_t, keep_rep)
                        else:
                            nc.vector.tensor_mul(wm_t, wm_t, keep_rep)
                        wm[(b2, n2)] = wm_t

                # ---- softmax denominators (PE broadcast) + attn ----
                recip = {}
                for b2 in range(2):
                    wmsum = p_wmsum.tile([128, H * NQ], bf16, tag="wmsum")
                    nc.vector.tensor_add(wmsum, wm[(b2, 0)], wm[(b2, 1)])
                    ps_s = pp.tile([128, H * NQ], f32, tag="sums", name="ps_s", bufs=1)
                    nc.tensor.matmul(
                        ps_s,
                        lhsT=ones_sb,
                        rhs=wmsum,
                        start=True,
                        stop=True,
                    )
                    r_sb = p_recip.tile([128, H * NQ], f32, tag="recip", name="r_sb")
                    nc.vector.reciprocal_approx_fast(out=r_sb, in_=ps_s)
                    recip[b2] = r_sb

                ps_a = pp.tile([128, 512], f32, tag="attn", name="ps_a")
                for hp in range(4):
                    for b2 in range(2):
                        for h2 in range(2):
                            h = 2 * hp + h2
                            col = (hp * 2 + b2) * 64
                            for n2 in range(2):
                                nc.tensor.matmul(
                                    ps_a[h2 * 64:(h2 + 1) * 64, col:col + 64],
                                    lhsT=v_sb[:, n2, b2, h * 64:(h + 1) * 64],
                                    rhs=wm[(b2, n2)][:, h * 64:(h + 1) * 64],
                                    start=(n2 == 0),
                                    stop=(n2 == 1),
                                )

                # normalize -> attn_sb (bf16), layout [128(2-head rows), (hp, b2, q)]
                attn_sb = p_attn.tile([128, 512], bf16, tag="attn_sb")
                for b2 in range(2):
                    for h2 in range(2):
                        rows = slice(h2 * 64, (h2 + 1) * 64)
                        o_ap = attn_sb[rows, :].rearrange(
                            "p (hp b q) -> p hp b q", hp=4, b=2)[:, :, b2, :]
                        i_ap = ps_a[rows, :].rearrange(
                            "p (hp b q) -> p hp b q", hp=4, b=2)[:, :, b2, :]
                        r_ap = recip[b2][rows, :].rearrange(
                            "p (hp x) -> p hp x", hp=4)[:, :, h2 * 64:(h2 + 1) * 64]
                        nc.vector.tensor_mul(o_ap, i_ap, r_ap)

                # ---- output projection + bias + post mask ----
                ps_o = pp.tile([128, OUT_DIM], f32, tag="attn", name="ps_o")
                for t in range(4):
                    nc.tensor.matmul(
                        ps_o,
                        lhsT=attn_sb[:, t * 128:(t + 1) * 128],
                        rhs=wo_sb[:, t, :],
                        start=(t == 0),
                        stop=(t == 3),
                    )
                out_sb = p_out.tile([128, OUT_DIM], f32, tag="out_sb")
                nc.vector.tensor_scalar_mul(
                    out_sb, in0=ps_o, scalar1=postm_sb[:, gpair:gpair + 1])
                nc.vector.scalar_tensor_tensor(
                    out_sb, in0=bias_bc,
                    scalar=postm_sb[:, gpair:gpair + 1],
                    in1=out_sb,
                    op0=mybir.AluOpType.mult, op1=mybir.AluOpType.add)
                nc.sync.dma_start(
                    out=out_d[gpair * 2:gpair * 2 + 2].rearrange("b q d -> (b q) d"),
                    in_=out_sb,
                )
    nc.compile()
    return nc


def _prep_inputs(entities, pre_mask, post_mask, W_in, W_out, b_out):
    """Host-side sharding + layout transforms (not part of timed HW work)."""
    # [oct, 128p, b, k, ne] contiguous per partition row
    eT = entities.reshape(BS // OCT, OCT, NE, 4, 128).transpose(
        0, 4, 1, 3, 2).astype(BF16)
    eT = np.ascontiguousarray(eT)
    keep = (1 - pre_mask).astype(np.float32).reshape(
        BS // OCT, OCT, NQ, 2, 128).transpose(0, 4, 1, 3, 2).astype(BF16)
    keep = np.ascontiguousarray(keep)
    kpost = (1 - post_mask).astype(np.float32)  # [BS, NQ]
    w_in = np.ascontiguousarray(W_in).reshape(4, 128, 3 * ED).astype(BF16)
    w_out = np.ascontiguousarray(W_out).reshape(4, 128, OUT_DIM).astype(BF16)
    b_o = b_out.reshape(1, OUT_DIM).astype(np.float32)

    in_maps = []
    for c in range(NCORES):
        sl = slice(c * BPC, (c + 1) * BPC)
        # postm: [128 rows=(b2*64+q), NPAIRS]
        kp = kpost[sl].reshape(NPAIRS, 128).T.copy()
        in_maps.append({
            "eT": np.ascontiguousarray(eT[c * NOCT:(c + 1) * NOCT]),
            "keep": np.ascontiguousarray(keep[c * NOCT:(c + 1) * NOCT]),
            "postm": np.ascontiguousarray(kp),
            "w_in": w_in,
            "w_out": w_out,
            "b_out": b_o,
        })
    return in_maps


def kernel(entities, pre_mask, post_mask, W_in, W_out, b_out, trace=False):
    global LAST_RESULT
    from concourse.bass_utils import run_bass_kernel_spmd

    if "nc" not in _BUILT:
        _BUILT["nc"] = _build_nc()
    nc = _BUILT["nc"]

    in_maps = _prep_inputs(np.asarray(entities), np.asarray(pre_mask),
                           np.asarray(post_mask), np.asarray(W_in),
                           np.asarray(W_out), np.asarray(b_out))
    res = run_bass_kernel_spmd(nc, in_maps, core_ids=list(range(NCORES)),
                               trace=trace)
    LAST_RESULT = res
    out = np.concatenate([r["out"] for r in res.results], axis=0)
    return out.astype(np.float32)


# revision 10
# speedup vs baseline: 1.5197x; 1.0887x over previous
"""Trainium2 Bass kernel for nn_EntityAttentionLayer (sparse attention).

Strategy (8 cores, data-parallel over bs):
  - Host side: shard bs across 8 cores (64 items each), pre-transpose
    entities to E^T[in_dim, ne] per batch, cast operands to bf16,
    convert masks to multiplicative keep-masks.
  - On chip, per batch b (processed in pairs, Q in octets of 8):
      K^T[ed, ne]  = (Wk^T E^T)        via lhsT=Wk slices, rhs=E^T
      V[ne, ed]    = E V-proj          via lhsT=E^T slices, rhs=Wv
      Q^T[ed, q]   =                   via lhsT=Wq slices, rhs=E^T[:, :64]
      logits^T[ne, q] per head         lhsT=K^T_h, rhs=Q^T_h  (ne on partitions)
      wm = exp(logits * 1/sqrt(hd))    on ACT (scale folded into activation)
      wm *= keep^T                     on DVE (multiplicative mask, h-broadcast)
      wmsum = wm(n2=0)+wm(n2=1)        on GPSIMD (halves the PE sums matmuls)
      sums broadcast [128, h*q]        single PE matmul with all-ones lhsT
      attn^T unnorm [2-heads, hp*b*q]  lhsT=V slices, rhs=wm  (col-tiled pairs)
      attn = attn_unnorm * 1/sums      DVE (approx reciprocal + strided muls)
      out[b*q, out] = attn^T.T @ W_out (+bias & post-mask fused on DVE)
      out *= keep_post (per-partition scalar), DMA out.
  All matmuls bf16 operands, fp32 PSUM accumulation.
"""

import numpy as np
import ml_dtypes

BS, NE, NQ, IN_DIM, ED, OUT_DIM, H, HD = 512, 256, 64, 512, 512, 512, 8, 64
NCORES = 8
BPC = BS // NCORES          # 64 batches per core
OCT = 8                     # batches per super-batch (Q^T amortization)
NOCT = BPC // OCT           # 8
PAIRS_PER_OCT = OCT // 2    # 4
NPAIRS = BPC // 2           # 32
SCALE = 1.0 / float(np.sqrt(HD))

BF16 = ml_dtypes.bfloat16

_BUILT = {}
LAST_RESULT = None


def _build_nc():
    import concourse.tile as tile
    from concourse import bacc, mybir
    from contextlib import ExitStack

    f32 = mybir.dt.float32
    bf16 = mybir.dt.bfloat16

    nc = bacc.Bacc("TRN2", target_bir_lowering=False)

    eT_d = nc.dram_tensor("eT", [NOCT, 128, OCT, 4, NE], bf16, kind="ExternalInput")
    keep_d = nc.dram_tensor("keep", [NOCT, 128, OCT, 2, NQ], bf16, kind="ExternalInput")
    postm_d = nc.dram_tensor("postm", [128, NPAIRS], f32, kind="ExternalInput")
    w_in_d = nc.dram_tensor("w_in", [4, 128, 3 * ED], bf16, kind="ExternalInput")
    w_out_d = nc.dram_tensor("w_out", [4, 128, OUT_DIM], bf16, kind="ExternalInput")
    b_out_d = nc.dram_tensor("b_out", [1, OUT_DIM], f32, kind="ExternalInput")
    out_d = nc.dram_tensor("out", [BPC, NQ, OUT_DIM], f32, kind="ExternalOutput")

    with ExitStack() as ctx:
        tc = ctx.enter_context(tile.TileContext(nc))
        consts = ctx.enter_context(tc.tile_pool(name="consts", bufs=1))
        p_eT = ctx.enter_context(tc.tile_pool(name="p_eT", bufs=3))
        p_keep = ctx.enter_context(tc.tile_pool(name="p_keep", bufs=3))
        p_kT = ctx.enter_context(tc.tile_pool(name="p_kT", bufs=3))
        p_v = ctx.enter_context(tc.tile_pool(name="p_v", bufs=3))
        p_wm = ctx.enter_context(tc.tile_pool(name="p_wm", bufs=8))
        p_wmsum = ctx.enter_context(tc.tile_pool(name="p_wmsum", bufs=3))
        p_recip = ctx.enter_context(tc.tile_pool(name="p_recip", bufs=2))
        p_attn = ctx.enter_context(tc.tile_pool(name="p_attn", bufs=2))
        p_out = ctx.enter_context(tc.tile_pool(name="p_out", bufs=3))
        pp = ctx.enter_context(tc.tile_pool(name="pp", bufs=2, space="PSUM"))

        # Constants
        w_sb = consts.tile([128, 4, 3 * ED], bf16)
        for wc, eng in ((0, nc.scalar), (1, nc.gpsimd), (2, nc.scalar)):
            eng.dma_start(
                out=w_sb[:, :, wc * ED:(wc + 1) * ED],
                in_=w_in_d[:, :, wc * ED:(wc + 1) * ED].rearrange("k p n -> p k n"))
        wo_sb = consts.tile([128, 4, OUT_DIM], bf16)
        nc.gpsimd.dma_start(out=wo_sb, in_=w_out_d[:, :, :].rearrange("k p n -> p k n"))
        bias_bc = consts.tile([128, OUT_DIM], f32)
        nc.gpsimd.dma_start(out=bias_bc, in_=b_out_d[:, :].to_broadcast([128, OUT_DIM]))
        postm_sb = consts.tile([128, NPAIRS], f32)
        nc.gpsimd.dma_start(out=postm_sb, in_=postm_d[:, :])
        ones_sb = consts.tile([128, 128], bf16)
        nc.vector.memset(ones_sb, 1.0)
        # Persistent zero-padded Q^T tiles (manual double buffer by octet
        # parity). Layout [128, m, h2, b, q]: head parity h2 selects which
        # 64-row half holds data; the other half stays zero so logits
        # matmuls can use full K=128 operands at base partition 0
        # (operands at base partition 64 fault on HW).
        qz0 = consts.tile([128, 4, 2, OCT, HD], bf16)
        nc.vector.memset(qz0, 0.0)
        qz1 = consts.tile([128, 4, 2, OCT, HD], bf16)
        nc.vector.memset(qz1, 0.0)
        qz_bufs = [qz0, qz1]

        def emit_tail(st):
            """sums + attn + out-proj for a finished pair (lagged one pair so
            the exp/keep/wmsum chain hides under the next pair's K/V)."""
            wm, v_sb, gpair = st["wm"], st["v_sb"], st["gpair"]
            recip = {}
            for b2 in range(2):
                wmsum = p_wmsum.tile([128, H * NQ], bf16, tag="wmsum")
                nc.vector.tensor_add(wmsum, wm[(b2, 0)], wm[(b2, 1)])
                ps_s = pp.tile([128, H * NQ], f32, tag="sums", name="ps_s", bufs=1)
                nc.tensor.matmul(
                    ps_s,
                    lhsT=ones_sb,
                    rhs=wmsum,
                    start=True,
                    stop=True,
                )
                r_sb = p_recip.tile([128, H * NQ], f32, tag="recip", name="r_sb")
                nc.vector.reciprocal_approx_fast(out=r_sb, in_=ps_s)
                recip[b2] = r_sb

            ps_a = pp.tile([128, 512], f32, tag="attn", name="ps_a")
            for hp in range(4):
                for b2 in range(2):
                    for h2 in range(2):
                        h = 2 * hp + h2
                        col = (hp * 2 + b2) * 64
                        for n2 in range(2):
                            nc.tensor.matmul(
                                ps_a[h2 * 64:(h2 + 1) * 64, col:col + 64],
                                lhsT=v_sb[:, n2, b2, h * 64:(h + 1) * 64],
                                rhs=wm[(b2, n2)][:, h * 64:(h + 1) * 64],
                                start=(n2 == 0),
                                stop=(n2 == 1),
                            )

            # normalize -> attn_sb (bf16), layout [128(2-head rows), (hp, b2, q)]
            attn_sb = p_attn.tile([128, 512], bf16, tag="attn_sb")
            for b2 in range(2):
                for h2 in range(2):
                    rows = slice(h2 * 64, (h2 + 1) * 64)
                    o_ap = attn_sb[rows, :].rearrange(
                        "p (hp b q) -> p hp b q", hp=4, b=2)[:, :, b2, :]
                    i_ap = ps_a[rows, :].rearrange(
                        "p (hp b q) -> p hp b q", hp=4, b=2)[:, :, b2, :]
                    r_ap = recip[b2][rows, :].rearrange(
                        "p (hp x) -> p hp x", hp=4)[:, :, h2 * 64:(h2 + 1) * 64]
                    nc.vector.tensor_mul(o_ap, i_ap, r_ap)

            # ---- output projection + bias + post mask ----
            ps_o = pp.tile([128, OUT_DIM], f32, tag="attn", name="ps_o")
            for t in range(4):
                nc.tensor.matmul(
                    ps_o,
                    lhsT=attn_sb[:, t * 128:(t + 1) * 128],
                    rhs=wo_sb[:, t, :],
                    start=(t == 0),
                    stop=(t == 3),
                )
            out_sb = p_out.tile([128, OUT_DIM], f32, tag="out_sb")
            nc.vector.tensor_scalar_mul(
                out_sb, in0=ps_o, scalar1=postm_sb[:, gpair:gpair + 1])
            nc.vector.scalar_tensor_tensor(
                out_sb, in0=bias_bc,
                scalar=postm_sb[:, gpair:gpair + 1],
                in1=out_sb,
                op0=mybir.AluOpType.mult, op1=mybir.AluOpType.add)
            nc.sync.dma_start(
                out=out_d[gpair * 2:gpair * 2 + 2].rearrange("b q d -> (b q) d"),
                in_=out_sb,
            )

        prev = None
        for oc in range(NOCT):
            # Pair-aligned DMA chunks: the K-proj of pair p depends only on
            # its own 2-batch slice, so compute can start on chunk 0.
            eT_sb = p_eT.tile([128, OCT, 4, NE], bf16, tag="eT")
            for pc, eng in ((0, nc.sync), (1, nc.scalar), (2, nc.sync),
                            (3, nc.scalar)):
                eng.dma_start(
                    out=eT_sb[:, pc * 2:(pc + 1) * 2, :, :],
                    in_=eT_d[oc, :, pc * 2:(pc + 1) * 2, :, :],
                )
            keep_sb = p_keep.tile([128, OCT, 2, NQ], bf16, tag="keep")
            nc.sync.dma_start(out=keep_sb, in_=keep_d[oc, :, :, :, :])
            qz = qz_bufs[oc % 2]

            for pr in range(PAIRS_PER_OCT):
                lb = pr * 2          # local batch index in octet
                gpair = oc * PAIRS_PER_OCT + pr

                # ---- K^T ----
                kT_sb = p_kT.tile([128, 4, 2, NE], bf16, tag="kT")
                for m in range(4):
                    ps_k = pp.tile([128, 2 * NE], f32, tag="proj", name="ps_k", bufs=3)
                    for k in range(4):
                        nc.tensor.matmul(
                            ps_k,
                            lhsT=w_sb[:, k, ED + m * 128:ED + (m + 1) * 128],
                            rhs=eT_sb[:, lb:lb + 2, k, :],
                            start=(k == 0),
                            stop=(k == 3),
                        )
                    if m % 2 == 0:
                        nc.vector.tensor_copy(out=kT_sb[:, m, :, :], in_=ps_k)
                    else:
                        nc.scalar.copy(out=kT_sb[:, m, :, :], in_=ps_k)

                # ---- V ----
                v_sb = p_v.tile([128, 2, 2, ED], bf16, tag="v")
                for n2 in range(2):
                    for b2 in range(2):
                        ps_v = pp.tile([128, ED], f32, tag="proj", name="ps_v", bufs=3)
                        for k in range(4):
                            nc.tensor.matmul(
                                ps_v,
                                lhsT=eT_sb[:, lb + b2, k, n2 * 128:(n2 + 1) * 128],
                                rhs=w_sb[:, k, 2 * ED:3 * ED],
                                start=(k == 0),
                                stop=(k == 3),
                            )
                        nc.scalar.copy(out=v_sb[:, n2, b2, :], in_=ps_v)

                # ---- Q^T for the whole octet (after first pair's K/V so the
                # ramp isn't blocked waiting for the full octet DMA) ----
                if pr == 0:
                    for m in range(4):
                        ps_q = pp.tile([128, OCT * HD], f32, tag="proj",
                                       name="ps_q", bufs=3)
                        for k in range(4):
                            nc.tensor.matmul(
                                ps_q,
                                lhsT=w_sb[:, k, m * 128:(m + 1) * 128],
                                rhs=eT_sb[:, :, k, 0:NQ],
                                start=(k == 0),
                                stop=(k == 3),
                            )
                        nc.scalar.copy(out=qz[0:64, m, 0, :, :], in_=ps_q[0:64, :])
                        nc.scalar.copy(out=qz[64:128, m, 1, :, :], in_=ps_q[64:128, :])

                # lagged tail of the previous pair fills the PE while this
                # pair's exp/keep chain drains on ACT/DVE/GPSIMD
                if prev is not None:
                    emit_tail(prev)

                # ---- logits^T + exp + keep-mask ----
                # wm[(b2, n2)] : [128(ne-slice), H*NQ] bf16
                wm = {}
                for n2 in range(2):
                    for b2 in range(2):
                        ps_l = pp.tile([128, H * NQ], f32, tag="logit", name="ps_l", bufs=2)
                        for hp in range(4):
                            # head pair shares the same K^T stationary slice;
                            # qz head-halves are complementarily zero-padded
                            nc.tensor.matmul(
                                ps_l[:, hp * 2 * NQ:(hp + 1) * 2 * NQ],
                                lhsT=kT_sb[:, hp, b2,
                                           n2 * 128:(n2 + 1) * 128],
                                rhs=qz[:, hp, :, lb + b2, :],
                                start=True,
                                stop=True,
                            )
                        wm_t = p_wm.tile([128, H * NQ], bf16, tag="wm", name="wm_t")
                        nc.scalar.activation(
                            out=wm_t, in_=ps_l,
                            func=mybir.ActivationFunctionType.Exp,
                            scale=SCALE,
                        )
                        keep_rep = keep_sb[:, lb + b2, n2, None, :].broadcast_to(
                            [128, H, NQ])
                        # n2=0 masks on the idle GPSIMD; n2=1 on DVE so the
                        # wmsum add right after it runs in-order (no hop)
                        if n2 == 0:
                            nc.gpsimd.tensor_mul(wm_t, wm_t, keep_rep)
                        else:
                            nc.vector.tensor_mul(wm_t, wm_t, keep_rep)
                        wm[(b2, n2)] = wm_t

                prev = {"wm": wm, "v_sb": v_sb, "gpair": gpair}

        emit_tail(prev)
    nc.compile()
    return nc


def _prep_inputs(entities, pre_mask, post_mask, W_in, W_out, b_out):
    """Host-side sharding + layout transforms (not part of timed HW work)."""
    # [oct, 128p, b, k, ne] contiguous per partition row
    eT = entities.reshape(BS // OCT, OCT, NE, 4, 128).transpose(
        0, 4, 1, 3, 2).astype(BF16)
    eT = np.ascontiguousarray(eT)
    keep = (1 - pre_mask).astype(np.float32).reshape(
        BS // OCT, OCT, NQ, 2, 128).transpose(0, 4, 1, 3, 2).astype(BF16)
    keep = np.ascontiguousarray(keep)
    kpost = (1 - post_mask).astype(np.float32)  # [BS, NQ]
    w_in = np.ascontiguousarray(W_in).reshape(4, 128, 3 * ED).astype(BF16)
    w_out = np.ascontiguousarray(W_out).reshape(4, 128, OUT_DIM).astype(BF16)
    b_o = b_out.reshape(1, OUT_DIM).astype(np.float32)

    in_maps = []
    for c in range(NCORES):
        sl = slice(c * BPC, (c + 1) * BPC)
        # postm: [128 rows=(b2*64+q), NPAIRS]
        kp = kpost[sl].reshape(NPAIRS, 128).T.copy()
        in_maps.append({
            "eT": np.ascontiguousarray(eT[c * NOCT:(c + 1) * NOCT]),
            "keep": np.ascontiguousarray(keep[c * NOCT:(c + 1) * NOCT]),
            "postm": np.ascontiguousarray(kp),
            "w_in": w_in,
            "w_out": w_out,
            "b_out": b_o,
        })
    return in_maps


def kernel(entities, pre_mask, post_mask, W_in, W_out, b_out, trace=False):
    global LAST_RESULT
    from concourse.bass_utils import run_bass_kernel_spmd

    if "nc" not in _BUILT:
        _BUILT["nc"] = _build_nc()
    nc = _BUILT["nc"]

    in_maps = _prep_inputs(np.asarray(entities), np.asarray(pre_mask),
                           np.asarray(post_mask), np.asarray(W_in),
                           np.asarray(W_out), np.asarray(b_out))
    res = run_bass_kernel_spmd(nc, in_maps, core_ids=list(range(NCORES)),
                               trace=trace)
    LAST_RESULT = res
    out = np.concatenate([r["out"] for r in res.results], axis=0)
    return out.astype(np.float32)
